# revision 51
# baseline (speedup 1.0000x reference)
"""Trainium2 Bass kernel for GCE-GNN LocalAggregator (gnn_message_passing).

Computes, for each batch b:
    h = embedding[inputs]                            # [N, D] gather
    e_k = leakyrelu((h * a_k) @ h.T, 0.2)            # k = 0..3
    alpha = softmax(where(adj == k+1, e_k, -inf))    # edge-type select
    out = alpha @ h

Sharding: data-parallel over batch B=512 across 8 cores (64 batches/core).

Structure (v4):
  * Pair layout: two batches share the 128 partitions ((u,i), u in {0,1}).
  * Gather via dma_gather (InstDMAGatherAnt) on FOUR parallel SWDGE
    queues - the gather ucode runs on Q7 cpu pair (2q, 2q+1), so four
    queues quadruple descriptor-gen throughput (the real bottleneck,
    ~8.4ns/row). int16 index limit beaten by gathering 512B dual rows
    from the table viewed as [V/2, 2*D] (idx>>1 <= 21548) and picking
    the half on-chip (copy + copy_predicated on the lo bit).
  * hT via PE transposes into per-half-chunk PSUM tiles (one batched
    evacuation per 4 pairs).
  * e matmuls write BF16 straight to PSUM; the edge-type select
    (copy_predicated with host-transposed one-hot masks) reads PSUM
    directly. e_k is symmetric in (i,j): only transposed attention is
    ever materialized.
  * exp(leakyrelu(x)): leakyrelu = max(0.2*x, x) in one fused DVE op,
    then one ACT Exp.
  * Softmax normalization BEFORE the out-matmul: R = blockdiag-ones^T
    @ xT gives every partition its row's sum (one matmul per chunk),
    then xTn = xT * reciprocal(R). Replaces 32 scaled evacuations + 32
    reciprocals with 1 matmul + 1 reciprocal + 1 multiply per chunk.
  * Output bf16 (tolerance 2e-2), upcast on host.
"""

import os
import sys

import numpy as np

for _p in ("/opt/trn_rl_repo",):
    if _p not in sys.path and os.path.isdir(_p):
        sys.path.insert(0, _p)

import ml_dtypes

import concourse.bass as bass
import concourse.bacc as bacc
import concourse.tile as tile
from concourse import mybir
from concourse.bass_utils import run_bass_kernel_spmd

B, N, D, V = 512, 64, 128, 43098
V2 = V // 2               # dual-row table height (43098 is even)
NCORES = 8
BC = B // NCORES          # 64 local batches per core
NPAIR = BC // 2           # 32 pairs
ALPHA = 0.2
NEG_BIG = -1.0e9          # exp(NEG_BIG) == 0; stands in for -9e15
CH = 16                   # pairs per compute chunk
NCH = NPAIR // CH         # 2 chunks
HH = 4                    # pairs per quarter-chunk (PSUM granularity)
GCH = 8                   # pairs per gather (decoupled from CH)

FP32 = mybir.dt.float32
BF16 = mybir.dt.bfloat16
I32 = mybir.dt.int32
I16 = mybir.dt.int16
AF = mybir.ActivationFunctionType
OP = mybir.AluOpType

BF = ml_dtypes.bfloat16


def build_nc():
    nc = bacc.Bacc(
        "TRN2", target_bir_lowering=False, debug=False, num_swdge_queues=4
    )

    emb2_d = nc.dram_tensor("emb2", [V2, 2 * D], BF16, kind="ExternalInput")
    idx16_d = nc.dram_tensor("idx16", [128, NPAIR * 8], I16, kind="ExternalInput")
    lo_d = nc.dram_tensor("lo", [128, NPAIR * D], mybir.dt.uint8, kind="ExternalInput")
    ident_d = nc.dram_tensor("ident", [128, 128], BF16, kind="ExternalInput")
    bdone_d = nc.dram_tensor("bdone", [128, 128], BF16, kind="ExternalInput")
    mt_d = nc.dram_tensor("mt", [128, 4 * NPAIR * N], mybir.dt.uint8, kind="ExternalInput")
    attn_d = nc.dram_tensor("attnT", [D, 4], FP32, kind="ExternalInput")
    out_d = nc.dram_tensor("out", [NPAIR, 128, D], BF16, kind="ExternalOutput")

    with tile.TileContext(nc) as tc:
        with (
            tc.tile_pool(name="singles", bufs=1) as singles,
            tc.tile_pool(name="big", bufs=1) as big,
            tc.tile_pool(name="chnk", bufs=3) as chnk,
            tc.tile_pool(name="outp", bufs=4) as outp,
            tc.tile_pool(name="ps_e", bufs=2, space="PSUM") as ps_e,
            tc.tile_pool(name="ps_o", bufs=2, space="PSUM") as ps_o,
            tc.tile_pool(name="ps_t", bufs=1, space="PSUM") as ps_t,
            tc.tile_pool(name="ps_r", bufs=1, space="PSUM") as ps_r,
        ):
            # ---- tiny inputs (idx first: the gathers gate on it) ----
            idx16_sb = singles.tile([128, NPAIR * 8], I16)
            nc.sync.dma_start(out=idx16_sb[:, :], in_=idx16_d[:, :])
            lo_sb = singles.tile([128, NPAIR, D], mybir.dt.uint8)
            nc.sync.dma_start(
                out=lo_sb[:, :, :].rearrange("p g d -> p (g d)"),
                in_=lo_d[:, :],
            )
            attn_sb = singles.tile([128, 4], FP32)
            nc.sync.dma_start(out=attn_sb[:, :], in_=attn_d[:, :])
            ident = singles.tile([128, 128], BF16)
            nc.sync.dma_start(out=ident[:, :], in_=ident_d[:, :])
            bdone = singles.tile([128, 128], BF16)
            nc.sync.dma_start(out=bdone[:, :], in_=bdone_d[:, :])

            # transposed edge-type masks [p, k, g, i] uint8
            mt_sb = big.tile([128, 4, NPAIR, N], mybir.dt.uint8, tag="mt")
            nc.scalar.dma_start(
                out=mt_sb[:, :, :, :].rearrange("p k g j -> p (k g j)"),
                in_=mt_d[:, :],
            )

            # hp2: gathered dual rows; hp: selected h; hT; S
            hp2 = big.tile([128, NPAIR, 2 * D], BF16, tag="hp2")
            hp = big.tile([128, NPAIR, D], BF16, tag="hp")
            hT = big.tile([128, NPAIR, 128], BF16, tag="hT")
            S_all = big.tile([128, 4, NPAIR, 128], BF16, tag="S")

            flat = "p g j -> p (g j)"
            chunk_state = {}

            # ---- all four gathers up front, one per SWDGE queue (the
            # gather ucode runs on Q7 cpu pair (2q, 2q+1): four queues
            # generate descriptors concurrently, ~4x throughput) ----
            for q in range(NPAIR // GCH):
                gs = slice(q * GCH, (q + 1) * GCH)
                nc.gpsimd.dma_gather(
                    out_ap=hp2[:, gs, :],
                    in_ap=emb2_d.ap(),
                    idxs_ap=idx16_sb[:, q * GCH * 8 : (q + 1) * GCH * 8],
                    num_idxs=GCH * 128,
                    num_idxs_reg=GCH * 128,
                    elem_size=2 * D,
                    queue_num=q,
                )

            def emit_ingest(c):
                """Half-select + PE transposes + S for chunk c."""
                g0 = c * CH
                gs = slice(g0, g0 + CH)
                # pick the right 256B half by the lo bit of the index
                nc.scalar.copy(out=hp[:, gs, :], in_=hp2[:, gs, 0:D])
                nc.vector.copy_predicated(
                    out=hp[:, gs, :],
                    mask=lo_sb[:, gs, :],
                    data=hp2[:, gs, D : 2 * D],
                )
                for h in range(CH // HH):
                    gh0 = g0 + h * HH
                    t4 = ps_t.tile([128, HH, 128], BF16, tag="t4")
                    for gl in range(HH):
                        nc.tensor.transpose(
                            out=t4[:, gl, :],
                            in_=hp[:, gh0 + gl, :],
                            identity=ident[:, :],
                        )
                    if h % 2 == 0:
                        nc.scalar.copy(
                            out=hT[:, gh0 : gh0 + HH, :], in_=t4[:, :, :]
                        )
                    else:
                        nc.vector.tensor_copy(
                            out=hT[:, gh0 : gh0 + HH, :], in_=t4[:, :, :]
                        )
                for k in range(4):
                    if k == 0:
                        nc.vector.tensor_scalar_mul(
                            out=S_all[:, k, gs, :].rearrange("p g q -> p (g q)"),
                            in0=hT[:, gs, :].rearrange("p g q -> p (g q)"),
                            scalar1=attn_sb[:, k : k + 1],
                        )
                    else:
                        nc.scalar.activation(
                            out=S_all[:, k, gs, :].rearrange("p g q -> p (g q)"),
                            in_=hT[:, gs, :].rearrange("p g q -> p (g q)"),
                            func=AF.Copy,
                            scale=attn_sb[:, k : k + 1],
                        )

            def emit_emm_select(c):
                """e matmuls (bf16 in PSUM) + select + lrelu/exp + norm."""
                g0 = c * CH
                alT = chnk.tile([128, CH, N], BF16, tag="alT")
                xTa = chnk.tile([128, CH, N], BF16, tag="xTa")
                xT = chnk.tile([128, CH, N], BF16, tag="xT")
                xTn = chnk.tile([128, CH, N], BF16, tag="xTn")
                rinv = chnk.tile([128, CH, N], FP32, tag="rinv")
                chunk_state[c] = xTn

                nc.gpsimd.memset(alT[:, :, :], NEG_BIG)

                for h in range(CH // HH):
                    gh0 = g0 + h * HH
                    # e_h[p=(u,i), pair, k, j] fp32 - lives only in PSUM
                    e_h = ps_e.tile([128, HH, 4, N], FP32, tag="e_h")
                    for gl in range(HH):
                        g = gh0 + gl
                        for u in range(2):
                            nc.tensor.matmul(
                                out=e_h[u * 64 : (u + 1) * 64, gl, :, :],
                                lhsT=hT[:, g, u * 64 : (u + 1) * 64],
                                rhs=S_all[:, :, g, u * 64 : (u + 1) * 64],
                                start=True,
                                stop=True,
                            )
                    # transposed select over the NEG background, straight
                    # from PSUM (e_k symmetric => same bytes serve the
                    # [(v,j), g, i] view)
                    hs = slice(h * HH, (h + 1) * HH)
                    for k in range(4):
                        nc.vector.copy_predicated(
                            out=alT[:, hs, :],
                            mask=mt_sb[:, k, gh0 : gh0 + HH, :],
                            data=e_h[:, :, k, :],
                        )

                # xT = exp(leakyrelu(.)); NEG entries give exact 0 through
                # exp. leakyrelu(x) = max(0.2*x, x) in one fused DVE op.
                nc.vector.scalar_tensor_tensor(
                    out=xTa[:, :, :],
                    in0=alT[:, :, :],
                    scalar=ALPHA,
                    in1=alT[:, :, :],
                    op0=OP.mult,
                    op1=OP.max,
                )
                nc.scalar.activation(
                    out=xT[:, :, :].rearrange(flat),
                    in_=xTa[:, :, :].rearrange(flat),
                    func=AF.Exp,
                )
                # normalization: R[p,(g,i)] = per-batch row sums via
                # blockdiag-ones matmul (every partition of half u gets
                # batch 2g+u's row sum), then xTn = xT / R.
                for h in range(2):
                    hs = slice(h * (CH // 2), (h + 1) * (CH // 2))
                    r_ps = ps_r.tile([128, CH // 2, N], FP32, tag="r_ps")
                    nc.tensor.matmul(
                        out=r_ps[:, :, :],
                        lhsT=bdone[:, :],
                        rhs=xT[:, hs, :],
                        start=True,
                        stop=True,
                    )
                    nc.vector.reciprocal_approx_fast(
                        out=rinv[:, hs, :].rearrange(flat),
                        in_=r_ps[:, :, :].rearrange("p g j -> p (g j)"),
                    )
                nc.vector.tensor_mul(
                    out=xTn[:, :, :], in0=xT[:, :, :], in1=rinv[:, :, :]
                )

            def emit_out(c):
                """out matmuls on pre-normalized xTn + evac + DMA."""
                xTn = chunk_state.pop(c)
                g0 = c * CH
                for h in range(CH // HH):
                    gh0 = g0 + h * HH
                    o4 = ps_o.tile([128, HH, D], FP32, tag="o4")
                    for gl in range(HH):
                        g = gh0 + gl
                        for u in range(2):
                            nc.tensor.matmul(
                                out=o4[u * 64 : (u + 1) * 64, gl, :],
                                lhsT=xTn[u * 64 : (u + 1) * 64, g - g0, :],
                                rhs=hp[u * 64 : (u + 1) * 64, g, :],
                                start=True,
                                stop=True,
                            )
                    o_sb = outp.tile([128, HH, D], BF16, tag="o_sb")
                    if h % 2 == 0:
                        nc.scalar.copy(out=o_sb[:, :, :], in_=o4[:, :, :])
                    else:
                        nc.vector.tensor_copy(out=o_sb[:, :, :], in_=o4[:, :, :])
                    nc.sync.dma_start(
                        out=out_d.ap()[gh0 : gh0 + HH, :, :].rearrange(
                            "g p d -> p g d"
                        ),
                        in_=o_sb[:, :, :],
                    )

            # Software pipeline: keep the PE queue fed.
            emit_ingest(0)
            emit_ingest(1)
            emit_emm_select(0)
            for c in range(1, NCH):
                if c + 1 < NCH:
                    emit_ingest(c + 1)
                emit_emm_select(c)
                emit_out(c - 1)
            emit_out(NCH - 1)
    nc.compile()
    return nc


_CACHE = {}


def _compiled():
    if "nc" not in _CACHE:
        _CACHE["nc"] = build_nc()
    return _CACHE["nc"]


def _shard_inputs(inputs, adj, embedding, attn_a):
    inputs = np.asarray(inputs)
    adj = np.asarray(adj)
    emb16 = np.asarray(embedding, dtype=np.float32).astype(BF)
    emb2 = np.ascontiguousarray(emb16.reshape(V2, 2 * D))
    attnT = np.ascontiguousarray(np.asarray(attn_a, dtype=np.float32).T)  # [D, 4]
    ident = np.ascontiguousarray(np.eye(128).astype(BF))
    bdone = np.zeros((128, 128), dtype=BF)
    bdone[:64, :64] = 1
    bdone[64:, 64:] = 1
    in_maps = []
    for c in range(NCORES):
        sl = slice(c * BC, (c + 1) * BC)
        # idx[(u,i), g] = inputs[c*BC + 2g+u, i]
        idx = np.ascontiguousarray(
            inputs[sl].reshape(NPAIR, 2, N).transpose(1, 2, 0).reshape(128, NPAIR)
            .astype(np.int32)
        )
        # dma_gather wants flat order i = g*128 + p, 16-partition-wrapped:
        # idx16[p, s] = hi[s*16 + p%16]
        flat = idx.T.reshape(-1).astype(np.int32)          # [4096] = (g, p)
        hi = (flat >> 1).astype(np.int16)
        lo = (flat & 1).astype(np.uint8)
        idx16 = np.ascontiguousarray(
            np.tile(hi.reshape(NPAIR * 8, 16).T, (8, 1))    # [128, 256]
        )
        lo_m = np.ascontiguousarray(
            np.repeat(lo.reshape(NPAIR, 128).T[:, :, None], D, axis=2)
            .reshape(128, NPAIR * D)
        )  # [p, g, d] expanded
        adj_r = adj[sl].reshape(NPAIR, 2, N, N).astype(np.int32)  # [g, u, i, j]
        Bm = adj_r.transpose(1, 3, 0, 2).reshape(128, NPAIR, N)  # [(v,j), g, i]
        # transposed one-hot edge-type masks [p, k, g, i] uint8
        mt = np.ascontiguousarray(
            np.stack([Bm == k + 1 for k in range(4)], axis=1)
            .astype(np.uint8).reshape(128, 4 * NPAIR * N)
        )
        in_maps.append(dict(emb2=emb2, idx16=idx16, lo=lo_m, ident=ident,
                            bdone=bdone, mt=mt, attnT=attnT))
    return in_maps


def kernel(inputs, adj, mask_item, item, embedding, attn_a):
    in_maps = _shard_inputs(inputs, adj, embedding, attn_a)
    res = run_bass_kernel_spmd(
        _compiled(), in_maps, core_ids=list(range(NCORES))
    ).results
    # out[g, p=(u,i), d] -> [b, i, d]: row b*N+i = 128*g + p
    out = np.concatenate(
        [np.asarray(res[c]["out"]).reshape(BC, N, D) for c in range(NCORES)],
        axis=0,
    )
    return out.astype(np.float32)
